# revision 1
# baseline (speedup 1.0000x reference)
"""BatchMultiHeadGraphAttention kernel for TRN2 (8 NeuronCores).

Reference computation (per graph b):
  h_prime = h @ w[head]                 [n, fo] per head
  t = tanh(h_prime)
  src[n] = t @ a_src[head];  dst[n] = t @ a_dst[head]
  s[i, j] = leaky_relu(src[i] + dst[j], 0.2)
  s masked where ~(adj | I); softmax over j; out = p @ h_prime + bias

Sharding: data-parallel over batch — one graph per core (BS=8, 8 cores).

Per-core layout strategy (scores kept transposed so the second matmul
contracts over the partition axis):
  - src/dst node scores via hpT = w.T @ h per head (PE), tanh (ACT), then a
    k=64 PE matmul against [a_src|a_dst] -> (src; dst) rows directly.
  - sT[j, i] = dst_j + src_i via a k=2 PE matmul into PSUM (bf16 operands).
  - Leaky: ACT Prelu(alpha=0.2) PSUM->SBUF bf16 for 5 of 8 j-blocks; DVE
    (0.2*x then max(5*x, x)) for the other 3 — balances the two engines.
  - Exp on ACT (2 instrs/head, bf16 out); adjacency mask = bf16 DVE multiply
    with valid^T built by u8->bf16 convert (Pool) + DMA transpose.
  - out[i, o] = sum_j p[j, i] * hp[j, o]: p chunks stationary with stride-8
    column interleave so the output DMA gets 2KB-contiguous descriptors; a
    ones column in hp yields the softmax denominator Z from the same matmul;
    bias is pre-added to hp (valid because sum_j p/Z = 1).
"""

import sys

import numpy as np

try:
    import concourse.bass  # noqa: F401
except ImportError:
    sys.path.insert(0, "/opt/trn_rl_repo")

BS, N, H, FI, FO = 8, 1024, 8, 256, 64
P = 128
NB = N // P     # 8 node blocks
FC = FI // P    # 2 f_in chunks
ALPHA = 0.2


def build_bass():
    import concourse.bass as bass
    import concourse.mybir as mybir
    from concourse import bacc
    from concourse.masks import make_identity
    from concourse.tile import TileContext

    f32 = mybir.dt.float32
    bf16 = mybir.dt.bfloat16
    u8 = mybir.dt.uint8
    AF = mybir.ActivationFunctionType
    OP = mybir.AluOpType
    AX = mybir.AxisListType

    nc = bacc.Bacc(trn_type="TRN2")

    h_d = nc.dram_tensor("h", [N, FI], f32, kind="ExternalInput")
    adj_d = nc.dram_tensor("adj", [N, N], u8, kind="ExternalInput")
    w_d = nc.dram_tensor("w", [H, FI, FO], f32, kind="ExternalInput")
    asrc_d = nc.dram_tensor("a_src", [H, FO], f32, kind="ExternalInput")
    adst_d = nc.dram_tensor("a_dst", [H, FO], f32, kind="ExternalInput")
    bias_d = nc.dram_tensor("bias", [FO], f32, kind="ExternalInput")
    out_d = nc.dram_tensor("out", [H, N, FO], f32, kind="ExternalOutput")

    with TileContext(nc) as tc:
        with (
            tc.tile_pool(name="singles", bufs=1) as singles,
            tc.tile_pool(name="temps", bufs=2) as temps,
            tc.tile_pool(name="ppool", bufs=2) as ppool,
            tc.tile_pool(name="spool", bufs=2) as spool,
        ):
            # ---------- long-lived tiles ----------
            ident = singles.tile([P, P], f32)
            make_identity(nc, ident)

            hT_sb = singles.tile([P, FC, N], bf16)  # [f_local, fc, n]
            # score-matmul operands: sd2 row0 = dstrow per head, row1 = ones
            #                        os2 row0 = ones, row1 = srcrow per head
            sd2 = singles.tile([2, H, N], bf16)
            os2 = singles.tile([2, H, N], bf16)
            ones_small = singles.tile([P, FO], bf16)
            nc.vector.memset(ones_small, 1.0)
            # replicate ones into sd2 row1 / os2 row0 (DMA; data rows DMA'd later)
            nc.sync.dma_start(out=sd2[1:2], in_=ones_small)
            nc.sync.dma_start(out=os2[0:1], in_=ones_small)

            # hp with ones column (col FO) and bias pre-added; bf16 for matmul
            hp_all = singles.tile([P, NB, H, FO + 1], bf16)
            nc.vector.memset(hp_all[:, :, :, FO : FO + 1], 1.0)

            vT = singles.tile([P, NB, N], bf16)  # valid^T: [j_local, jb, i]
            identb = singles.tile([P, P], bf16)
            nc.gpsimd.tensor_copy(out=identb, in_=ident)

            # ================= phase A: prep =================
            with (
                tc.tile_pool(name="phA", bufs=1) as phA,
                tc.tile_pool(name="tempA", bufs=4) as tempA,
                tc.tile_pool(name="psumA", bufs=1, space="PSUM") as psA,
                tc.tile_pool(name="psumHP", bufs=1, space="PSUM") as psHP,
                tc.tile_pool(name="psumR", bufs=2, space="PSUM") as psR,
            ):
                w_sb = phA.tile([P, FC, H, FO], bf16)
                w_re = w_d.rearrange("h (c p) o -> p c h o", p=P)
                for fc in range(FC):
                    nc.gpsimd.dma_start(out=w_sb[:, fc], in_=w_re[:, fc])

                # a_src/a_dst: load as [16, 64], transpose to [64, (vec h)]
                a2d = phA.tile([2 * H, FO], f32)
                nc.sync.dma_start(out=a2d[0:H], in_=asrc_d[:, :])
                nc.sync.dma_start(out=a2d[H : 2 * H], in_=adst_d[:, :])
                pa2 = psR.tile([FO, 2 * H], f32, tag="hpT")
                nc.tensor.transpose(pa2, a2d, ident[0 : 2 * H, 0 : 2 * H])
                a2T = phA.tile([FO, 2 * H], bf16)
                nc.vector.tensor_copy(out=a2T, in_=pa2)
                bias_b = phA.tile([P, FO], f32)
                nc.sync.dma_start(
                    out=bias_b,
                    in_=bass.AP(tensor=bias_d, offset=0, ap=[[0, P], [1, FO]]),
                )

                h_sb = phA.tile([P, NB, FI], bf16)
                h_re = h_d.rearrange("(nb p) f -> p nb f", p=P)
                for nb in range(NB):
                    nc.gpsimd.dma_start(out=h_sb[:, nb], in_=h_re[:, nb])

                for nb in range(NB):
                    for fc in range(FC):
                        pt = psA.tile([P, P], bf16, tag="tr")
                        nc.tensor.transpose(
                            pt, h_sb[:, nb, fc * P : (fc + 1) * P], identb
                        )
                        nc.scalar.copy(
                            out=hT_sb[:, fc, nb * P : (nb + 1) * P], in_=pt
                        )

                # ----- src/dst rows via transposed h_prime (per head) -----
                # hpT[o, n] = sum_f w[f, o] h[n, f]; tanhT; then a2T.T @ tanhT
                # gives (src; dst) rows directly in row layout.
                sdrows = phA.tile([2, H, N], bf16)
                for hd in range(H):
                    phT = psR.tile([FO, N], f32, tag="hpT")
                    for fc in range(FC):
                        for half in range(2):
                            nc.tensor.matmul(
                                phT[:, half * 512 : (half + 1) * 512],
                                lhsT=w_sb[:, fc, hd, :],
                                rhs=hT_sb[:, fc, half * 512 : (half + 1) * 512],
                                start=(fc == 0),
                                stop=(fc == FC - 1),
                                skip_group_check=True,
                            )
                    tT = tempA.tile([FO, N], bf16, tag="tT")
                    nc.scalar.activation(out=tT, in_=phT, func=AF.Tanh)
                    a2_ap = bass.AP(
                        tensor=a2T.tensor,
                        offset=a2T.offset + hd,
                        ap=[list(a2T.ap[0]), [H, 2]],
                    )
                    psd = psHP.tile([2, N], f32, tag="sd")
                    for half in range(2):
                        nc.tensor.matmul(
                            psd[:, half * 512 : (half + 1) * 512],
                            lhsT=a2_ap,
                            rhs=tT[:, half * 512 : (half + 1) * 512],
                            start=True,
                            stop=True,
                        )
                    nc.vector.tensor_copy(out=sdrows[:, hd, :], in_=psd)

                # row 0 = srcrow per head; row 1 = dstrow per head
                nc.sync.dma_start(out=os2[1:2], in_=sdrows[0:1])
                nc.sync.dma_start(out=sd2[0:1], in_=sdrows[1:2])

                # ----- hp (+bias, bf16) for the output matmul -----
                for nb in range(NB):
                    php = psHP.tile([P, H, FO], f32, tag="hp")
                    for fc in range(FC):
                        nc.tensor.matmul(
                            php,
                            lhsT=hT_sb[:, fc, nb * P : (nb + 1) * P],
                            rhs=w_sb[:, fc],
                            start=(fc == 0),
                            stop=(fc == FC - 1),
                        )
                    bias_rep = bass.AP(
                        tensor=bias_b.tensor,
                        offset=bias_b.offset,
                        ap=[list(bias_b.ap[0]), [0, H], list(bias_b.ap[1])],
                    )
                    nc.vector.tensor_add(
                        out=hp_all[:, nb, :, 0:FO], in0=php, in1=bias_rep
                    )

                # ----- adjacency: valid^T via DMA transpose, bf16 -----
                for ib in range(NB):
                    adj_u8 = tempA.tile([P, N], u8, tag="adj8")
                    nc.sync.dma_start(
                        out=adj_u8, in_=adj_d[ib * P : (ib + 1) * P, :]
                    )
                    adjb = tempA.tile([P, N], bf16, tag="adjb")
                    nc.gpsimd.tensor_copy(out=adjb, in_=adj_u8)
                    nc.sync.dma_start_transpose(
                        out=vT[:, :, ib * P : (ib + 1) * P], in_=adjb
                    )
                # diagonal always valid: OR identity onto diag blocks
                for jb in range(NB):
                    nc.vector.tensor_max(
                        out=vT[:, jb, jb * P : (jb + 1) * P],
                        in0=vT[:, jb, jb * P : (jb + 1) * P],
                        in1=identb,
                    )

            # ================= phase B: attention =================
            with (
                tc.tile_pool(name="psumS", bufs=3, space="PSUM") as psS,
                tc.tile_pool(name="psumO", bufs=1, space="PSUM") as psO,
            ):
                for hd in range(H):
                    s_sb = spool.tile([P, NB, N], bf16, tag="s")
                    for jb in range(NB):
                        ps = psS.tile([P, N], f32, tag="spre")
                        for half in range(2):
                            nc.tensor.matmul(
                                ps[:, half * 512 : (half + 1) * 512],
                                lhsT=sd2[:, hd, jb * P : (jb + 1) * P],
                                rhs=os2[:, hd, half * 512 : (half + 1) * 512],
                                start=True,
                                stop=True,
                            )
                        if jb >= 3:
                            nc.scalar.activation(
                                out=s_sb[:, jb, :], in_=ps, func=AF.Prelu,
                                alpha=ALPHA,
                            )
                        else:
                            # DVE leaky: s = 0.2*ps (bf16), then s = max(5*s, s)
                            nc.vector.tensor_scalar_mul(
                                out=s_sb[:, jb, :], in0=ps, scalar1=ALPHA
                            )
                            nc.vector.scalar_tensor_tensor(
                                out=s_sb[:, jb, :], in0=s_sb[:, jb, :],
                                scalar=5.0, in1=s_sb[:, jb, :],
                                op0=OP.mult, op1=OP.max,
                            )
                    p_sb = ppool.tile([P, NB, N], bf16, tag="p")
                    nc.scalar.activation(
                        out=p_sb[:, 0:4], in_=s_sb[:, 0:4], func=AF.Exp
                    )
                    nc.scalar.activation(
                        out=p_sb[:, 4:8], in_=s_sb[:, 4:8], func=AF.Exp
                    )
                    nc.vector.tensor_mul(out=p_sb, in0=p_sb, in1=vT)

                    po_a = psO.tile([P, 4, FO + 1], f32, tag="o2a")
                    po_b = psO.tile([P, 4, FO + 1], f32, tag="o2b")
                    for ic in range(NB):
                        po = po_a if ic < 4 else po_b
                        icl = ic % 4
                        for jb in range(NB):
                            lhsT_str = bass.AP(
                                tensor=p_sb.tensor,
                                offset=p_sb[:, jb, ic : ic + 1].offset,
                                ap=[list(p_sb.ap[0]), [NB, P]],
                            )
                            nc.tensor.matmul(
                                po[:, icl, :],
                                lhsT=lhsT_str,
                                rhs=hp_all[:, jb, hd, :],
                                start=(jb == 0),
                                stop=(jb == NB - 1),
                            )

                    rz = temps.tile([P, NB, 1], f32, tag="rz")
                    nc.vector.reciprocal(out=rz[:, 0:4], in_=po_a[:, :, FO : FO + 1])
                    nc.vector.reciprocal(out=rz[:, 4:8], in_=po_b[:, :, FO : FO + 1])
                    o_sb = temps.tile([P, NB, FO], f32, tag="osb")
                    rzb_a = bass.AP(
                        tensor=rz.tensor, offset=rz[:, 0:4, :].offset,
                        ap=[list(rz.ap[0]), [rz.ap[1][0], 4], [0, FO]],
                    )
                    rzb_b = bass.AP(
                        tensor=rz.tensor, offset=rz[:, 4:8, :].offset,
                        ap=[list(rz.ap[0]), [rz.ap[1][0], 4], [0, FO]],
                    )
                    nc.vector.tensor_mul(
                        out=o_sb[:, 0:4, :], in0=po_a[:, :, 0:FO], in1=rzb_a
                    )
                    nc.vector.tensor_mul(
                        out=o_sb[:, 4:8, :], in0=po_b[:, :, 0:FO], in1=rzb_b
                    )
                    out_eng = nc.gpsimd if hd % 2 == 0 else nc.sync
                    out_eng.dma_start(
                        out=out_d[hd].rearrange("(p ic) o -> p ic o", ic=NB),
                        in_=o_sb,
                    )
    nc.finalize()
    return nc


_NC_CACHE = None
TRACE = False
LAST_RESULT = None


def kernel(h, adj, w, a_src, a_dst, bias):
    global _NC_CACHE
    from concourse.bass_utils import run_bass_kernel_spmd

    if _NC_CACHE is None:
        _NC_CACHE = build_bass()
    nc = _NC_CACHE

    h = np.ascontiguousarray(np.asarray(h, dtype=np.float32))
    adj_u8 = np.ascontiguousarray(np.asarray(adj).astype(np.uint8))
    w = np.ascontiguousarray(np.asarray(w, dtype=np.float32))
    a_src2 = np.ascontiguousarray(np.asarray(a_src, dtype=np.float32)[..., 0])
    a_dst2 = np.ascontiguousarray(np.asarray(a_dst, dtype=np.float32)[..., 0])
    bias = np.ascontiguousarray(np.asarray(bias, dtype=np.float32))

    in_maps = [
        {
            "h": h[b],
            "adj": adj_u8[b],
            "w": w,
            "a_src": a_src2,
            "a_dst": a_dst2,
            "bias": bias,
        }
        for b in range(BS)
    ]
    res = run_bass_kernel_spmd(
        nc, in_maps, core_ids=list(range(BS)), trace=TRACE,
        trace_cores=list(range(BS)) if TRACE else None,
    )
    if TRACE:
        global LAST_RESULT
        LAST_RESULT = res
    out = np.stack([r["out"] for r in res.results], axis=0)
    return out.astype(np.float32)



# revision 34
# speedup vs baseline: 1.1332x; 1.1332x over previous
"""BatchMultiHeadGraphAttention kernel for TRN2 (8 NeuronCores).

Reference computation (per graph b):
  h_prime = h @ w[head]                 [n, fo] per head
  t = tanh(h_prime)
  src[n] = t @ a_src[head];  dst[n] = t @ a_dst[head]
  s[i, j] = leaky_relu(src[i] + dst[j], 0.2)
  s masked where ~(adj | I); softmax over j; out = p @ h_prime + bias

Sharding: data-parallel over batch — one graph per core (BS=8, 8 cores).

Per-core layout (scores transposed: s[j, i] so the output matmul contracts
over the partition axis). Engine budget balanced across PE/ACT/DVE/Pool:
  - maskT[j, i] = -144 where edge invalid (0 else), built from adj with one
    Pool tensor_scalar ((adj*144)-144) + DMA transpose; diagonal cleared by
    multiplying diag blocks with (1 - I).
  - Scores: k=2 PE matmul (dst_j + src_i). For most (head, jb) tiles the
    mask is folded in by a second accumulating PE matmul (ident @ maskT);
    the remaining tiles get a Pool bf16 add of maskT after the leaky
    (exp turns -144 into 0 either way).
  - leaky = (0.2*s) max s: ONE scalar_tensor_tensor from PSUM, statically
    split between DVE and ACT (Prelu); exp on ACT (2 instrs/head).
  - src/dst node scores: heads processed in PAIRS so the transposed
    h_prime matmul uses all 128 partitions (halves tanh cost); the
    (src|dst) row matmul writes a bf16 PSUM tile, evicted at DVE 2x and
    scattered into the (dst;1)/(1;src) score-matmul operands via
    SBUF->SBUF DMA.
  - out[i, o] = sum_j p[j, i] * hp[j, o]: p chunks stationary with stride-8
    column interleave (2KB-contiguous output DMA); a ones column in hp
    yields the softmax denominator from the same matmul; bias pre-added.
"""

import sys

import numpy as np

try:
    import concourse.bass  # noqa: F401
except ImportError:
    sys.path.insert(0, "/opt/trn_rl_repo")

BS, N, H, FI, FO = 8, 1024, 8, 256, 64
P = 128
NB = N // P     # 8 node blocks
FC = FI // P    # 2 f_in chunks
HP = H // 2     # head pairs
ALPHA = 0.2
BIG = 144.0

# static per-(head, jb) tables:
#   leaky engine: 'A' = ACT Prelu, 'V' = DVE stt   (~16 ACT / 48 DVE)
#   mask path: True = fold into PSUM via PE matmul, False = Pool bf16 add
LEAKY_TAB = [[None] * NB for _ in range(H)]
MASK_TAB = [[True] * NB for _ in range(H)]
for _hd in range(H):
    for _jb in range(NB):
        k = _hd * NB + _jb
        LEAKY_TAB[_hd][_jb] = "A" if k % 2 == 0 else "V"
        MASK_TAB[_hd][_jb] = True


def build_bass():
    import concourse.bass as bass
    import concourse.mybir as mybir
    from concourse import bacc
    from concourse.masks import make_identity
    from concourse.tile import TileContext

    f32 = mybir.dt.float32
    bf16 = mybir.dt.bfloat16
    u8 = mybir.dt.uint8
    AF = mybir.ActivationFunctionType
    OP = mybir.AluOpType

    nc = bacc.Bacc(trn_type="TRN2")

    h_d = nc.dram_tensor("h", [N, FI], f32, kind="ExternalInput")
    adj_d = nc.dram_tensor("adj", [N, N], u8, kind="ExternalInput")
    w_d = nc.dram_tensor("w", [H, FI, FO], f32, kind="ExternalInput")
    asrc_d = nc.dram_tensor("a_src", [H, FO], f32, kind="ExternalInput")
    adst_d = nc.dram_tensor("a_dst", [H, FO], f32, kind="ExternalInput")
    bias_d = nc.dram_tensor("bias", [FO], f32, kind="ExternalInput")
    out_d = nc.dram_tensor("out", [H, N, FO], f32, kind="ExternalOutput")

    with TileContext(nc) as tc:
        with (
            tc.tile_pool(name="singles", bufs=1) as singles,
            tc.tile_pool(name="temps", bufs=2) as temps,
            tc.tile_pool(name="ppool", bufs=2) as ppool,
            tc.tile_pool(name="spool", bufs=2) as spool,
        ):
            # ---------- long-lived tiles ----------
            ident = singles.tile([P, P], f32)
            make_identity(nc, ident)

            hT_sb = singles.tile([P, FC, N], bf16)  # [f_local, fc, n]
            # score-matmul operands: sd2 row0 = dstrow per head, row1 = ones
            #                        os2 row0 = ones, row1 = srcrow per head
            sd2 = singles.tile([2, H, N], bf16)
            os2 = singles.tile([2, H, N], bf16)
            ones_small = singles.tile([P, FO], bf16)
            nc.vector.memset(ones_small, 1.0)
            nc.gpsimd.dma_start(out=sd2[1:2], in_=ones_small)
            nc.gpsimd.dma_start(out=os2[0:1], in_=ones_small)

            # hp with ones column (col FO) and bias pre-added; bf16 for matmul
            hp_all = singles.tile([P, NB, H, FO + 1], bf16)
            nc.vector.memset(hp_all[:, :, :, FO : FO + 1], 1.0)

            # maskT[j_local, jb, i] = -144 invalid, 0 valid
            maskT = singles.tile([P, NB, N], bf16)
            identb = singles.tile([P, P], bf16)
            nc.gpsimd.tensor_copy(out=identb, in_=ident)
            # 1 - I for clearing the diagonal of maskT
            om_ident = singles.tile([P, P], bf16)
            nc.vector.tensor_scalar(
                out=om_ident, in0=identb, scalar1=-1.0, scalar2=1.0,
                op0=OP.mult, op1=OP.add,
            )

            # ================= phase A: prep =================
            with (
                tc.tile_pool(name="phA", bufs=1) as phA,
                tc.tile_pool(name="tempA", bufs=4) as tempA,
                tc.tile_pool(name="adjp", bufs=8) as adjp,
                tc.tile_pool(name="psumA", bufs=2, space="PSUM") as psA,
                tc.tile_pool(name="psumHP", bufs=2, space="PSUM") as psHP,
                tc.tile_pool(name="psumR", bufs=2, space="PSUM") as psR,
                tc.tile_pool(name="psumD", bufs=1, space="PSUM") as psD,
            ):
                # bulk loads: adjacency via one HWDGE (sync) u8 DMA so it
                # lands early without eating SWDGE ring space; h/w via
                # casting gpsimd SWDGE DMAs (ring-sized halves).
                adj_sb = phA.tile([P, NB, N], u8)
                adj_re = adj_d.rearrange("(ib p) j -> p ib j", p=P)
                nc.sync.dma_start(out=adj_sb, in_=adj_re)

                h_sb = phA.tile([P, NB, FI], bf16)
                h_re = h_d.rearrange("(nb p) f -> p nb f", p=P)
                nc.gpsimd.dma_start(out=h_sb[:, 0:4], in_=h_re[:, 0:4])
                nc.gpsimd.dma_start(out=h_sb[:, 4:8], in_=h_re[:, 4:8])
                w_sb = phA.tile([P, FC, H, FO], bf16)
                w_re = w_d.rearrange("h (c p) o -> p c h o", p=P)
                for fc in range(FC):
                    nc.gpsimd.dma_start(out=w_sb[:, fc], in_=w_re[:, fc])

                # a_src/a_dst: load as [16, 64], transpose to [64, 16]
                a2d = phA.tile([2 * H, FO], f32)
                nc.sync.dma_start(out=a2d[0:H], in_=asrc_d[:, :])
                nc.sync.dma_start(out=a2d[H : 2 * H], in_=adst_d[:, :])

                bias_b = phA.tile([P, FO], f32)
                nc.sync.dma_start(
                    out=bias_b,
                    in_=bass.AP(tensor=bias_d, offset=0, ap=[[0, P], [1, FO]]),
                )
                # adjacency -> vT (valid {0,1}, diag forced valid), then
                # maskT = 144*vT - 144 in one DVE 4x pass per i-block.
                # u8 -> bf16 converts split Pool (6) / DVE (2).
                for ib in range(NB):
                    adjm = adjp.tile([P, N], bf16, tag="adjm")
                    conv = nc.vector if ib >= 6 else nc.gpsimd
                    conv.tensor_copy(out=adjm, in_=adj_sb[:, ib])
                    tr_eng = nc.sync if ib < 4 else nc.scalar
                    tr_eng.dma_start_transpose(
                        out=maskT[:, :, ib * P : (ib + 1) * P], in_=adjm
                    )
                    # diagonal always valid
                    nc.vector.tensor_max(
                        out=maskT[:, ib, ib * P : (ib + 1) * P],
                        in0=maskT[:, ib, ib * P : (ib + 1) * P],
                        in1=identb,
                    )
                for jb in range(NB):
                    nc.vector.tensor_scalar(
                        out=maskT[:, jb, :], in0=maskT[:, jb, :],
                        scalar1=BIG, scalar2=-BIG, op0=OP.mult, op1=OP.add,
                    )

                pa2 = psR.tile([FO, 2 * H], f32, tag="hpT")
                nc.tensor.transpose(pa2, a2d, ident[0 : 2 * H, 0 : 2 * H])
                a2T = phA.tile([FO, 2 * H], bf16)
                nc.vector.tensor_copy(out=a2T, in_=pa2)
                # paired-head weight cols: a2p[:, m, q]; head 2q on partitions
                # 0-63, head 2q+1 on 64-127; m = (src, dst, src', dst')
                a2p = phA.tile([P, 4, HP], bf16)
                nc.vector.memset(a2p, 0.0)
                for q in range(HP):
                    # src_h at a2T col h, dst_h at col H+h
                    nc.vector.tensor_copy(
                        out=a2p[0:FO, 0:2, q],
                        in_=bass.AP(
                            tensor=a2T.tensor, offset=a2T.offset + 2 * q,
                            ap=[list(a2T.ap[0]), [H, 2]],
                        ),
                    )
                    nc.vector.tensor_copy(
                        out=a2p[FO : 2 * FO, 2:4, q],
                        in_=bass.AP(
                            tensor=a2T.tensor, offset=a2T.offset + 2 * q + 1,
                            ap=[list(a2T.ap[0]), [H, 2]],
                        ),
                    )

                for nb in range(NB):
                    for fc in range(FC):
                        pt = psA.tile([P, P], bf16, tag="tr")
                        nc.tensor.transpose(
                            pt, h_sb[:, nb, fc * P : (fc + 1) * P], identb
                        )
                        nc.vector.tensor_copy(
                            out=hT_sb[:, fc, nb * P : (nb + 1) * P], in_=pt
                        )

                # ----- src/dst rows via paired transposed h_prime -----
                # hpT pair q: partitions 0-63 = head 2q's o, 64-127 = 2q+1's
                # stage rows per pair: (src_2q, dst_2q, src_2q+1, dst_2q+1)
                stage = phA.tile([4, HP, N], bf16)
                for q in range(HP):
                    tTp = tempA.tile([P, N], bf16, tag="tT")
                    for half in range(2):
                        hs = slice(half * 512, (half + 1) * 512)
                        phT = psR.tile([P, 512], f32, tag="hpT")
                        for fc in range(FC):
                            nc.tensor.matmul(
                                phT,
                                lhsT=w_sb[:, fc, 2 * q : 2 * q + 2, :],
                                rhs=hT_sb[:, fc, hs],
                                start=(fc == 0),
                                stop=(fc == FC - 1),
                                skip_group_check=True,
                            )
                        nc.scalar.activation(
                            out=tTp[:, hs], in_=phT, func=AF.Tanh
                        )
                    psd = psD.tile([4, N], f32, tag="sd")
                    for half in range(2):
                        hs = slice(half * 512, (half + 1) * 512)
                        nc.tensor.matmul(
                            psd[:, hs], lhsT=a2p[:, :, q], rhs=tTp[:, hs],
                            start=True, stop=True,
                        )
                    nc.vector.tensor_copy(out=stage[:, q, :], in_=psd)
                    # scatter this pair: dsts (stage parts 1,3) -> sd2[0],
                    # srcs (parts 0,2) -> os2[1]; h = 2q + parity
                    pstride = HP * N
                    nc.gpsimd.dma_start(
                        out=sd2[0:1, 2 * q : 2 * q + 2, :],
                        in_=bass.AP(
                            tensor=stage.tensor,
                            offset=stage.offset + pstride + q * N,
                            ap=[[2 * pstride, 2], [1, N]],
                        ),
                    )
                    nc.gpsimd.dma_start(
                        out=os2[1:2, 2 * q : 2 * q + 2, :],
                        in_=bass.AP(
                            tensor=stage.tensor, offset=stage.offset + q * N,
                            ap=[[2 * pstride, 2], [1, N]],
                        ),
                    )


                # ----- hp (+bias, bf16) for the output matmul -----
                for nb in range(NB):
                    php = psHP.tile([P, H, FO], f32, tag="hp")
                    for fc in range(FC):
                        nc.tensor.matmul(
                            php,
                            lhsT=hT_sb[:, fc, nb * P : (nb + 1) * P],
                            rhs=w_sb[:, fc],
                            start=(fc == 0),
                            stop=(fc == FC - 1),
                        )
                    bias_rep = bass.AP(
                        tensor=bias_b.tensor,
                        offset=bias_b.offset,
                        ap=[list(bias_b.ap[0]), [0, H], list(bias_b.ap[1])],
                    )
                    nc.vector.tensor_add(
                        out=hp_all[:, nb, :, 0:FO], in0=php, in1=bias_rep
                    )

            # ================= phase B: attention =================
            # Software-pipelined over heads: iteration hd emits
            #   exp(hd-1) [ACT first so it never waits behind prelu(hd)],
            #   scores+leaky(hd) [PE + DVE/ACT/Pool],
            #   out-matmul + normalize + store for hd-1 [PE after scores(hd)].
            # This keeps the strict-FIFO PE queue from stalling on exp.
            with (
                tc.tile_pool(name="psumS", bufs=3, space="PSUM") as psS,
                tc.tile_pool(name="psumO", bufs=1, space="PSUM") as psO,
            ):
                def emit_scores(hd):
                    s_sb = spool.tile([P, NB, N], bf16, tag="s")
                    for jb in range(NB):
                        in_psum = MASK_TAB[hd][jb]
                        ps = psS.tile([P, N], f32, tag="spre")
                        for half in range(2):
                            hs = slice(half * 512, (half + 1) * 512)
                            nc.tensor.matmul(
                                ps[:, hs],
                                lhsT=sd2[:, hd, jb * P : (jb + 1) * P],
                                rhs=os2[:, hd, hs],
                                start=True,
                                stop=not in_psum,
                                skip_group_check=True,
                            )
                            if in_psum:
                                nc.tensor.matmul(
                                    ps[:, hs],
                                    lhsT=identb,
                                    rhs=maskT[:, jb, hs],
                                    start=False,
                                    stop=True,
                                    skip_group_check=True,
                                )
                        if LEAKY_TAB[hd][jb] == "A":
                            nc.scalar.activation(
                                out=s_sb[:, jb, :], in_=ps, func=AF.Prelu,
                                alpha=ALPHA,
                            )
                        else:
                            # HW allows one PSUM operand per instruction:
                            # evict alpha*s to SBUF, then max(5*t, t) there.
                            nc.vector.tensor_scalar_mul(
                                out=s_sb[:, jb, :], in0=ps, scalar1=ALPHA
                            )
                            nc.vector.scalar_tensor_tensor(
                                out=s_sb[:, jb, :], in0=s_sb[:, jb, :],
                                scalar=5.0, in1=s_sb[:, jb, :],
                                op0=OP.mult, op1=OP.max,
                            )
                    return s_sb

                def emit_exp(s_sb):
                    p_sb = ppool.tile([P, NB, N], bf16, tag="p")
                    nc.scalar.activation(
                        out=p_sb[:, 0:4], in_=s_sb[:, 0:4], func=AF.Exp
                    )
                    nc.scalar.activation(
                        out=p_sb[:, 4:8], in_=s_sb[:, 4:8], func=AF.Exp
                    )
                    return p_sb

                def emit_out(hd, p_sb):
                    po_a = psO.tile([P, 4, FO + 1], f32, tag="o2a")
                    po_b = psO.tile([P, 4, FO + 1], f32, tag="o2b")
                    for ic in range(NB):
                        po = po_a if ic < 4 else po_b
                        icl = ic % 4
                        for jb in range(NB):
                            lhsT_str = bass.AP(
                                tensor=p_sb.tensor,
                                offset=p_sb[:, jb, ic : ic + 1].offset,
                                ap=[list(p_sb.ap[0]), [NB, P]],
                            )
                            nc.tensor.matmul(
                                po[:, icl, :],
                                lhsT=lhsT_str,
                                rhs=hp_all[:, jb, hd, :],
                                start=(jb == 0),
                                stop=(jb == NB - 1),
                            )

                    rz = temps.tile([P, NB, 1], f32, tag="rz")
                    nc.vector.reciprocal(out=rz[:, 0:4], in_=po_a[:, :, FO : FO + 1])
                    nc.vector.reciprocal(out=rz[:, 4:8], in_=po_b[:, :, FO : FO + 1])
                    o_sb = temps.tile([P, NB, FO], f32, tag="osb")
                    rzb_a = bass.AP(
                        tensor=rz.tensor, offset=rz[:, 0:4, :].offset,
                        ap=[list(rz.ap[0]), [rz.ap[1][0], 4], [0, FO]],
                    )
                    rzb_b = bass.AP(
                        tensor=rz.tensor, offset=rz[:, 4:8, :].offset,
                        ap=[list(rz.ap[0]), [rz.ap[1][0], 4], [0, FO]],
                    )
                    nc.vector.tensor_mul(
                        out=o_sb[:, 0:4, :], in0=po_a[:, :, 0:FO], in1=rzb_a
                    )
                    nc.vector.tensor_mul(
                        out=o_sb[:, 4:8, :], in0=po_b[:, :, 0:FO], in1=rzb_b
                    )
                    out_eng = nc.gpsimd
                    out_eng.dma_start(
                        out=out_d[hd].rearrange("(p ic) o -> p ic o", ic=NB),
                        in_=o_sb,
                    )

                prev_s = None
                for hd in range(H):
                    if prev_s is not None:
                        p_prev = emit_exp(prev_s)
                    prev_s_new = emit_scores(hd)
                    if prev_s is not None:
                        emit_out(hd - 1, p_prev)
                    prev_s = prev_s_new
                p_last = emit_exp(prev_s)
                emit_out(H - 1, p_last)
    nc.finalize()
    return nc


_NC_CACHE = None
TRACE = False
LAST_RESULT = None


def kernel(h, adj, w, a_src, a_dst, bias):
    global _NC_CACHE
    from concourse.bass_utils import run_bass_kernel_spmd

    if _NC_CACHE is None:
        _NC_CACHE = build_bass()
    nc = _NC_CACHE

    h = np.ascontiguousarray(np.asarray(h, dtype=np.float32))
    adj_u8 = np.ascontiguousarray(np.asarray(adj).astype(np.uint8))
    w = np.ascontiguousarray(np.asarray(w, dtype=np.float32))
    a_src2 = np.ascontiguousarray(np.asarray(a_src, dtype=np.float32)[..., 0])
    a_dst2 = np.ascontiguousarray(np.asarray(a_dst, dtype=np.float32)[..., 0])
    bias = np.ascontiguousarray(np.asarray(bias, dtype=np.float32))

    in_maps = [
        {
            "h": h[b],
            "adj": adj_u8[b],
            "w": w,
            "a_src": a_src2,
            "a_dst": a_dst2,
            "bias": bias,
        }
        for b in range(BS)
    ]
    res = run_bass_kernel_spmd(
        nc, in_maps, core_ids=list(range(BS)), trace=TRACE,
        trace_cores=list(range(BS)) if TRACE else None,
    )
    if TRACE:
        global LAST_RESULT
        LAST_RESULT = res
    out = np.stack([r["out"] for r in res.results], axis=0)
    return out.astype(np.float32)


# revision 38
# speedup vs baseline: 1.1736x; 1.0356x over previous
"""BatchMultiHeadGraphAttention kernel for TRN2 (8 NeuronCores).

Reference computation (per graph b):
  h_prime = h @ w[head]                 [n, fo] per head
  t = tanh(h_prime)
  src[n] = t @ a_src[head];  dst[n] = t @ a_dst[head]
  s[i, j] = leaky_relu(src[i] + dst[j], 0.2)
  s masked where ~(adj | I); softmax over j; out = p @ h_prime + bias

Sharding: data-parallel over batch — one graph per core (BS=8, 8 cores).

Per-core layout (scores transposed: s[j, i] so the output matmul contracts
over the partition axis). Engine budget balanced across PE/ACT/DVE/Pool:
  - maskT[j, i] = -144 where edge invalid (0 else), built from adj with one
    Pool tensor_scalar ((adj*144)-144) + DMA transpose; diagonal cleared by
    multiplying diag blocks with (1 - I).
  - Scores: k=2 PE matmul (dst_j + src_i). For most (head, jb) tiles the
    mask is folded in by a second accumulating PE matmul (ident @ maskT);
    the remaining tiles get a Pool bf16 add of maskT after the leaky
    (exp turns -144 into 0 either way).
  - leaky = (0.2*s) max s: ONE scalar_tensor_tensor from PSUM, statically
    split between DVE and ACT (Prelu); exp on ACT (2 instrs/head).
  - src/dst node scores: heads processed in PAIRS so the transposed
    h_prime matmul uses all 128 partitions (halves tanh cost); the
    (src|dst) row matmul writes a bf16 PSUM tile, evicted at DVE 2x and
    scattered into the (dst;1)/(1;src) score-matmul operands via
    SBUF->SBUF DMA.
  - out[i, o] = sum_j p[j, i] * hp[j, o]: p chunks stationary with stride-8
    column interleave (2KB-contiguous output DMA); a ones column in hp
    yields the softmax denominator from the same matmul; bias pre-added.
"""

import sys

import numpy as np

try:
    import concourse.bass  # noqa: F401
except ImportError:
    sys.path.insert(0, "/opt/trn_rl_repo")

BS, N, H, FI, FO = 8, 1024, 8, 256, 64
P = 128
NB = N // P     # 8 node blocks
FC = FI // P    # 2 f_in chunks
HP = H // 2     # head pairs
ALPHA = 0.2
BIG = 144.0

# static per-(head, jb) tables:
#   leaky engine: 'A' = ACT Prelu, 'V' = DVE stt   (~16 ACT / 48 DVE)
#   mask path: True = fold into PSUM via PE matmul, False = Pool bf16 add
LEAKY_TAB = [[None] * NB for _ in range(H)]
MASK_TAB = [[True] * NB for _ in range(H)]
for _hd in range(H):
    for _jb in range(NB):
        k = _hd * NB + _jb
        LEAKY_TAB[_hd][_jb] = "A" if k % 2 == 0 else "V"
        MASK_TAB[_hd][_jb] = True


def build_bass():
    import concourse.bass as bass
    import concourse.mybir as mybir
    from concourse import bacc
    from concourse.masks import make_identity
    from concourse.tile import TileContext

    f32 = mybir.dt.float32
    bf16 = mybir.dt.bfloat16
    u8 = mybir.dt.uint8
    AF = mybir.ActivationFunctionType
    OP = mybir.AluOpType

    nc = bacc.Bacc(trn_type="TRN2")

    h_d = nc.dram_tensor("h", [N, FI], f32, kind="ExternalInput")
    adj_d = nc.dram_tensor("adj", [N, N], u8, kind="ExternalInput")
    w_d = nc.dram_tensor("w", [H, FI, FO], f32, kind="ExternalInput")
    asrc_d = nc.dram_tensor("a_src", [H, FO], f32, kind="ExternalInput")
    adst_d = nc.dram_tensor("a_dst", [H, FO], f32, kind="ExternalInput")
    bias_d = nc.dram_tensor("bias", [FO], f32, kind="ExternalInput")
    out_d = nc.dram_tensor("out", [H, N, FO], f32, kind="ExternalOutput")

    with TileContext(nc) as tc:
        with (
            tc.tile_pool(name="singles", bufs=1) as singles,
            tc.tile_pool(name="temps", bufs=2) as temps,
            tc.tile_pool(name="ppool", bufs=2) as ppool,
            tc.tile_pool(name="spool", bufs=2) as spool,
        ):
            # ---------- long-lived tiles ----------
            ident = singles.tile([P, P], f32)
            make_identity(nc, ident)

            hT_sb = singles.tile([P, FC, N], bf16)  # [f_local, fc, n]
            # score-matmul operands: sd2 row0 = dstrow per head, row1 = ones
            #                        os2 row0 = ones, row1 = srcrow per head
            sd2 = singles.tile([2, H, N], bf16)
            os2 = singles.tile([2, H, N], bf16)
            ones_small = singles.tile([P, FO], bf16)
            nc.vector.memset(ones_small, 1.0)
            nc.gpsimd.dma_start(out=sd2[1:2], in_=ones_small)
            nc.gpsimd.dma_start(out=os2[0:1], in_=ones_small)

            # hp with ones column (col FO) and bias pre-added; bf16 for matmul
            hp_all = singles.tile([P, NB, H, FO + 1], bf16)
            nc.vector.memset(hp_all[:, :, :, FO : FO + 1], 1.0)

            # maskT[j_local, jb, i] = -144 invalid, 0 valid
            maskT = singles.tile([P, NB, N], bf16)
            identb = singles.tile([P, P], bf16)
            nc.gpsimd.tensor_copy(out=identb, in_=ident)
            # 1 - I for clearing the diagonal of maskT
            om_ident = singles.tile([P, P], bf16)
            nc.vector.tensor_scalar(
                out=om_ident, in0=identb, scalar1=-1.0, scalar2=1.0,
                op0=OP.mult, op1=OP.add,
            )

            # ================= phase A: prep =================
            with (
                tc.tile_pool(name="phA", bufs=1) as phA,
                tc.tile_pool(name="tempA", bufs=4) as tempA,
                tc.tile_pool(name="adjp", bufs=8) as adjp,
                tc.tile_pool(name="psumA", bufs=2, space="PSUM") as psA,
                tc.tile_pool(name="psumHP", bufs=2, space="PSUM") as psHP,
                tc.tile_pool(name="psumR", bufs=2, space="PSUM") as psR,
                tc.tile_pool(name="psumD", bufs=1, space="PSUM") as psD,
            ):
                # bulk loads: adjacency via one HWDGE (sync) u8 DMA so it
                # lands early without eating SWDGE ring space; h/w via
                # casting gpsimd SWDGE DMAs (ring-sized halves).
                adj_sb = phA.tile([P, NB, N], u8)
                adj_re = adj_d.rearrange("(ib p) j -> p ib j", p=P)
                nc.sync.dma_start(out=adj_sb, in_=adj_re)

                h_sb = phA.tile([P, NB, FI], bf16)
                h_re = h_d.rearrange("(nb p) f -> p nb f", p=P)
                nc.gpsimd.dma_start(out=h_sb[:, 0:4], in_=h_re[:, 0:4])
                nc.gpsimd.dma_start(out=h_sb[:, 4:8], in_=h_re[:, 4:8])
                w_sb = phA.tile([P, FC, H, FO], bf16)
                w_re = w_d.rearrange("h (c p) o -> p c h o", p=P)
                for fc in range(FC):
                    nc.gpsimd.dma_start(out=w_sb[:, fc], in_=w_re[:, fc])

                # a_src/a_dst: load as [16, 64], transpose to [64, 16]
                a2d = phA.tile([2 * H, FO], f32)
                nc.sync.dma_start(out=a2d[0:H], in_=asrc_d[:, :])
                nc.sync.dma_start(out=a2d[H : 2 * H], in_=adst_d[:, :])

                bias_b = phA.tile([P, FO], f32)
                nc.sync.dma_start(
                    out=bias_b,
                    in_=bass.AP(tensor=bias_d, offset=0, ap=[[0, P], [1, FO]]),
                )
                # adjacency -> vT (valid {0,1}, diag forced valid), then
                # maskT = 144*vT - 144 in one DVE 4x pass per i-block.
                # Converts: Pool (ib0-5) / DVE (ib6-7). Transposes ib0-5 on
                # the SP HWDGE queue here; ib6-7 are deferred onto the ACT
                # queue AFTER the tanh chain so they don't stall it.
                adjms = {}
                for ib in range(NB):
                    adjm = adjp.tile([P, N], bf16, tag="adjm")
                    adjms[ib] = adjm
                    conv = nc.vector if ib >= 6 else nc.gpsimd
                    conv.tensor_copy(out=adjm, in_=adj_sb[:, ib])
                    if ib < 6:
                        nc.sync.dma_start_transpose(
                            out=maskT[:, :, ib * P : (ib + 1) * P], in_=adjm
                        )
                        nc.vector.tensor_max(
                            out=maskT[:, ib, ib * P : (ib + 1) * P],
                            in0=maskT[:, ib, ib * P : (ib + 1) * P],
                            in1=identb,
                        )

                def finish_mask():
                    for ib in (6, 7):
                        nc.scalar.dma_start_transpose(
                            out=maskT[:, :, ib * P : (ib + 1) * P],
                            in_=adjms[ib],
                        )
                        nc.vector.tensor_max(
                            out=maskT[:, ib, ib * P : (ib + 1) * P],
                            in0=maskT[:, ib, ib * P : (ib + 1) * P],
                            in1=identb,
                        )
                    for jb in range(NB):
                        nc.vector.tensor_scalar(
                            out=maskT[:, jb, :], in0=maskT[:, jb, :],
                            scalar1=BIG, scalar2=-BIG,
                            op0=OP.mult, op1=OP.add,
                        )

                pa2 = psR.tile([FO, 2 * H], f32, tag="hpT")
                nc.tensor.transpose(pa2, a2d, ident[0 : 2 * H, 0 : 2 * H])
                a2T = phA.tile([FO, 2 * H], bf16)
                nc.vector.tensor_copy(out=a2T, in_=pa2)
                # paired-head weight cols: a2p[:, m, q]; head 2q on partitions
                # 0-63, head 2q+1 on 64-127; m = (src, dst, src', dst')
                a2p = phA.tile([P, 4, HP], bf16)
                nc.vector.memset(a2p, 0.0)
                for q in range(HP):
                    # src_h at a2T col h, dst_h at col H+h
                    nc.vector.tensor_copy(
                        out=a2p[0:FO, 0:2, q],
                        in_=bass.AP(
                            tensor=a2T.tensor, offset=a2T.offset + 2 * q,
                            ap=[list(a2T.ap[0]), [H, 2]],
                        ),
                    )
                    nc.vector.tensor_copy(
                        out=a2p[FO : 2 * FO, 2:4, q],
                        in_=bass.AP(
                            tensor=a2T.tensor, offset=a2T.offset + 2 * q + 1,
                            ap=[list(a2T.ap[0]), [H, 2]],
                        ),
                    )

                for nb in range(NB):
                    for fc in range(FC):
                        pt = psA.tile([P, P], bf16, tag="tr")
                        nc.tensor.transpose(
                            pt, h_sb[:, nb, fc * P : (fc + 1) * P], identb
                        )
                        nc.vector.tensor_copy(
                            out=hT_sb[:, fc, nb * P : (nb + 1) * P], in_=pt
                        )

                # ----- src/dst rows via paired transposed h_prime -----
                # hpT pair q: partitions 0-63 = head 2q's o, 64-127 = 2q+1's
                # stage rows per pair: (src_2q, dst_2q, src_2q+1, dst_2q+1)
                stage = phA.tile([4, HP, N], bf16)
                for q in range(HP):
                    tTp = tempA.tile([P, N], bf16, tag="tT")
                    for half in range(2):
                        hs = slice(half * 512, (half + 1) * 512)
                        phT = psR.tile([P, 512], f32, tag="hpT")
                        for fc in range(FC):
                            nc.tensor.matmul(
                                phT,
                                lhsT=w_sb[:, fc, 2 * q : 2 * q + 2, :],
                                rhs=hT_sb[:, fc, hs],
                                start=(fc == 0),
                                stop=(fc == FC - 1),
                                skip_group_check=True,
                            )
                        nc.scalar.activation(
                            out=tTp[:, hs], in_=phT, func=AF.Tanh
                        )
                    psd = psD.tile([4, N], f32, tag="sd")
                    for half in range(2):
                        hs = slice(half * 512, (half + 1) * 512)
                        nc.tensor.matmul(
                            psd[:, hs], lhsT=a2p[:, :, q], rhs=tTp[:, hs],
                            start=True, stop=True,
                        )
                    nc.vector.tensor_copy(out=stage[:, q, :], in_=psd)
                    # scatter this pair: dsts (stage parts 1,3) -> sd2[0],
                    # srcs (parts 0,2) -> os2[1]; h = 2q + parity
                    pstride = HP * N
                    nc.gpsimd.dma_start(
                        out=sd2[0:1, 2 * q : 2 * q + 2, :],
                        in_=bass.AP(
                            tensor=stage.tensor,
                            offset=stage.offset + pstride + q * N,
                            ap=[[2 * pstride, 2], [1, N]],
                        ),
                    )
                    nc.gpsimd.dma_start(
                        out=os2[1:2, 2 * q : 2 * q + 2, :],
                        in_=bass.AP(
                            tensor=stage.tensor, offset=stage.offset + q * N,
                            ap=[[2 * pstride, 2], [1, N]],
                        ),
                    )


                finish_mask()

                # ----- hp (+bias, bf16) for the output matmul -----
                for nb in range(NB):
                    php = psHP.tile([P, H, FO], f32, tag="hp")
                    for fc in range(FC):
                        nc.tensor.matmul(
                            php,
                            lhsT=hT_sb[:, fc, nb * P : (nb + 1) * P],
                            rhs=w_sb[:, fc],
                            start=(fc == 0),
                            stop=(fc == FC - 1),
                        )
                    bias_rep = bass.AP(
                        tensor=bias_b.tensor,
                        offset=bias_b.offset,
                        ap=[list(bias_b.ap[0]), [0, H], list(bias_b.ap[1])],
                    )
                    nc.vector.tensor_add(
                        out=hp_all[:, nb, :, 0:FO], in0=php, in1=bias_rep
                    )

            # ================= phase B: attention =================
            # Software-pipelined over heads: iteration hd emits
            #   exp(hd-1) [ACT first so it never waits behind prelu(hd)],
            #   scores+leaky(hd) [PE + DVE/ACT/Pool],
            #   out-matmul + normalize + store for hd-1 [PE after scores(hd)].
            # This keeps the strict-FIFO PE queue from stalling on exp.
            with (
                tc.tile_pool(name="psumS", bufs=3, space="PSUM") as psS,
                tc.tile_pool(name="psumO", bufs=1, space="PSUM") as psO,
            ):
                def emit_scores(hd):
                    s_sb = spool.tile([P, NB, N], bf16, tag="s")
                    for jb in range(NB):
                        in_psum = MASK_TAB[hd][jb]
                        ps = psS.tile([P, N], f32, tag="spre")
                        for half in range(2):
                            hs = slice(half * 512, (half + 1) * 512)
                            nc.tensor.matmul(
                                ps[:, hs],
                                lhsT=sd2[:, hd, jb * P : (jb + 1) * P],
                                rhs=os2[:, hd, hs],
                                start=True,
                                stop=not in_psum,
                                skip_group_check=True,
                            )
                            if in_psum:
                                nc.tensor.matmul(
                                    ps[:, hs],
                                    lhsT=identb,
                                    rhs=maskT[:, jb, hs],
                                    start=False,
                                    stop=True,
                                    skip_group_check=True,
                                )
                        if LEAKY_TAB[hd][jb] == "A":
                            nc.scalar.activation(
                                out=s_sb[:, jb, :], in_=ps, func=AF.Prelu,
                                alpha=ALPHA,
                            )
                        else:
                            # HW allows one PSUM operand per instruction:
                            # t = alpha*s (1x evict), r = 4*relu(t) (4x),
                            # s = t + r = leaky(s) (2x). Cheaper than the
                            # mode-less scalar_tensor_tensor finisher.
                            t1 = spool.tile([P, N], bf16, tag="t1")
                            nc.vector.tensor_scalar_mul(
                                out=t1, in0=ps, scalar1=ALPHA
                            )
                            r4 = spool.tile([P, N], bf16, tag="r4")
                            nc.vector.tensor_scalar(
                                out=r4, in0=t1, scalar1=0.0, scalar2=4.0,
                                op0=OP.max, op1=OP.mult,
                            )
                            nc.vector.tensor_add(
                                out=s_sb[:, jb, :], in0=t1, in1=r4
                            )
                    return s_sb

                def emit_exp(s_sb):
                    p_sb = ppool.tile([P, NB, N], bf16, tag="p")
                    nc.scalar.activation(
                        out=p_sb[:, 0:4], in_=s_sb[:, 0:4], func=AF.Exp
                    )
                    nc.scalar.activation(
                        out=p_sb[:, 4:8], in_=s_sb[:, 4:8], func=AF.Exp
                    )
                    return p_sb

                def emit_out(hd, p_sb):
                    po_a = psO.tile([P, 4, FO + 1], f32, tag="o2a")
                    po_b = psO.tile([P, 4, FO + 1], f32, tag="o2b")
                    for ic in range(NB):
                        po = po_a if ic < 4 else po_b
                        icl = ic % 4
                        for jb in range(NB):
                            lhsT_str = bass.AP(
                                tensor=p_sb.tensor,
                                offset=p_sb[:, jb, ic : ic + 1].offset,
                                ap=[list(p_sb.ap[0]), [NB, P]],
                            )
                            nc.tensor.matmul(
                                po[:, icl, :],
                                lhsT=lhsT_str,
                                rhs=hp_all[:, jb, hd, :],
                                start=(jb == 0),
                                stop=(jb == NB - 1),
                            )

                    rz = temps.tile([P, NB, 1], f32, tag="rz")
                    nc.vector.reciprocal(out=rz[:, 0:4], in_=po_a[:, :, FO : FO + 1])
                    nc.vector.reciprocal(out=rz[:, 4:8], in_=po_b[:, :, FO : FO + 1])
                    o_sb = temps.tile([P, NB, FO], f32, tag="osb")
                    rzb_a = bass.AP(
                        tensor=rz.tensor, offset=rz[:, 0:4, :].offset,
                        ap=[list(rz.ap[0]), [rz.ap[1][0], 4], [0, FO]],
                    )
                    rzb_b = bass.AP(
                        tensor=rz.tensor, offset=rz[:, 4:8, :].offset,
                        ap=[list(rz.ap[0]), [rz.ap[1][0], 4], [0, FO]],
                    )
                    nc.vector.tensor_mul(
                        out=o_sb[:, 0:4, :], in0=po_a[:, :, 0:FO], in1=rzb_a
                    )
                    nc.vector.tensor_mul(
                        out=o_sb[:, 4:8, :], in0=po_b[:, :, 0:FO], in1=rzb_b
                    )
                    out_eng = nc.gpsimd
                    out_eng.dma_start(
                        out=out_d[hd].rearrange("(p ic) o -> p ic o", ic=NB),
                        in_=o_sb,
                    )

                prev_s = None
                for hd in range(H):
                    if prev_s is not None:
                        p_prev = emit_exp(prev_s)
                    prev_s_new = emit_scores(hd)
                    if prev_s is not None:
                        emit_out(hd - 1, p_prev)
                    prev_s = prev_s_new
                p_last = emit_exp(prev_s)
                emit_out(H - 1, p_last)
    nc.finalize()
    return nc


_NC_CACHE = None
TRACE = False
LAST_RESULT = None


def kernel(h, adj, w, a_src, a_dst, bias):
    global _NC_CACHE
    from concourse.bass_utils import run_bass_kernel_spmd

    if _NC_CACHE is None:
        _NC_CACHE = build_bass()
    nc = _NC_CACHE

    h = np.ascontiguousarray(np.asarray(h, dtype=np.float32))
    adj_u8 = np.ascontiguousarray(np.asarray(adj).astype(np.uint8))
    w = np.ascontiguousarray(np.asarray(w, dtype=np.float32))
    a_src2 = np.ascontiguousarray(np.asarray(a_src, dtype=np.float32)[..., 0])
    a_dst2 = np.ascontiguousarray(np.asarray(a_dst, dtype=np.float32)[..., 0])
    bias = np.ascontiguousarray(np.asarray(bias, dtype=np.float32))

    in_maps = [
        {
            "h": h[b],
            "adj": adj_u8[b],
            "w": w,
            "a_src": a_src2,
            "a_dst": a_dst2,
            "bias": bias,
        }
        for b in range(BS)
    ]
    res = run_bass_kernel_spmd(
        nc, in_maps, core_ids=list(range(BS)), trace=TRACE,
        trace_cores=list(range(BS)) if TRACE else None,
    )
    if TRACE:
        global LAST_RESULT
        LAST_RESULT = res
    out = np.stack([r["out"] for r in res.results], axis=0)
    return out.astype(np.float32)
